# revision 1
# baseline (speedup 1.0000x reference)
"""Llama GQA attention (B=1, S=2048, HID=4096, 32 Q heads / 8 KV heads, RoPE,
causal) on 8 trn2 NeuronCores, tensor-parallel over KV heads.

Per core c: q-heads 4c..4c+3, kv-head c. Device computes a partial
out_c = attn_heads_c @ Wo[:, cols_c].T ; host sums the 8 partials.

Layout strategy (per core):
  - projections: QT/KT [d, s] via  W_chunk.T @ xT_chunk  (contraction over HID)
  - RoPE: half-swap via SBUF->SBUF DMA, sign folded into sin table
  - scores(T): ST[k,q] = KT_chunk.T @ QT  -> exp on ACT (no max-sub; values
    are small), causal: fully-masked k-chunks skipped, diagonal 512-blocks
    multiplied by a binary mask derived from the provided attention_mask
  - rowsum over k (partition dim) via ones-vector matmul; normalization is
    applied to attn_outT with a DMA-broadcast reciprocal
  - PV: attn_unnormT[d,q] = V_chunk.T(natural [k,d]) @ E_chunk, V obtained
    via PE transpose of the VT projection
  - Wo: out[s,e] = attnT_chunk.T @ WoT_chunk, fp32 partial out
All matmuls in bf16 with fp32 PSUM accumulation.
"""
import math

import numpy as np
import ml_dtypes

S = 2048
HID = 4096
D = 128
NQ = 4            # q heads per core
NCORES = 8
SB = 512          # s/q block
NSB = S // SB     # 4
NKC = S // D      # 16 k chunks
NEB = HID // 512  # 8 output e blocks
NCC = HID // D    # 32 contraction chunks
SCALE = 1.0 / math.sqrt(D)
ROPE_THETA = 10000.0

BF16 = ml_dtypes.bfloat16

_CACHE = {}


def _build():
    import concourse.tile as tile
    from concourse import bacc, mybir
    from concourse.masks import make_identity

    dt = mybir.dt
    nc = bacc.Bacc("TRN2", target_bir_lowering=False, debug=False,
                   num_devices=NCORES)

    xT = nc.dram_tensor("xT", [HID, S], dt.bfloat16, kind="ExternalInput")
    wqT = nc.dram_tensor("wqT", [HID, NQ * D], dt.bfloat16, kind="ExternalInput")
    wkT = nc.dram_tensor("wkT", [HID, D], dt.bfloat16, kind="ExternalInput")
    wvT = nc.dram_tensor("wvT", [HID, D], dt.bfloat16, kind="ExternalInput")
    woT = nc.dram_tensor("woT", [NQ * D, HID], dt.bfloat16, kind="ExternalInput")
    cosT = nc.dram_tensor("cosT", [D, S], dt.bfloat16, kind="ExternalInput")
    sinT = nc.dram_tensor("sinT", [D, S], dt.bfloat16, kind="ExternalInput")
    maskTd = nc.dram_tensor("maskTd", [NSB, SB, SB], dt.bfloat16,
                            kind="ExternalInput")
    part = nc.dram_tensor("part", [S, HID], dt.float32, kind="ExternalOutput")

    xTr = xT.rearrange("(ko p) s -> p ko s", p=D)                 # [128,32,2048]
    wqr = wqT.rearrange("(ko p) (h d) -> p ko h d", p=D, d=D)     # [128,32,4,128]
    wkr = wkT.rearrange("(ko p) d -> p ko d", p=D)                # [128,32,128]
    wvr = wvT.rearrange("(ko p) d -> p ko d", p=D)
    wor = woT.rearrange("(h p) (eb e) -> p h eb e", p=D, e=512)   # [128,4,8,512]
    maskr = maskTd.rearrange("j (kc p) q -> p j kc q", p=D)       # [128,4,4,512]

    with tile.TileContext(nc) as tc:
        _body(nc, tc, tile, mybir, make_identity,
              xTr, wqr, wkr, wvr, wor, maskr, cosT, sinT, part)
    nc.compile()
    return nc


def _body(nc, tc, tile, mybir, make_identity,
          xTr, wqr, wkr, wvr, wor, maskr, cosT, sinT, part):
    from contextlib import ExitStack

    dt = mybir.dt
    AF = mybir.ActivationFunctionType

    with ExitStack() as ctx:
        const = ctx.enter_context(tc.tile_pool(name="const", bufs=1))
        persist = ctx.enter_context(tc.tile_pool(name="persist", bufs=1))
        big = ctx.enter_context(tc.tile_pool(name="big", bufs=3))
        wpool = ctx.enter_context(tc.tile_pool(name="w", bufs=2))
        tr = ctx.enter_context(tc.tile_pool(name="tr", bufs=2))
        outp = ctx.enter_context(tc.tile_pool(name="outp", bufs=2))
        ps = ctx.enter_context(tc.tile_pool(name="ps", bufs=6, space="PSUM"))
        drp = ctx.enter_context(tc.tile_pool(name="drp", bufs=2, space="DRAM"))

        # ---- constants / persistent tensors ----
        mask = const.tile([D, NSB, 4, SB], dt.bfloat16)
        nc.sync.dma_start(out=mask, in_=maskr)
        ones = const.tile([D, 1], dt.bfloat16)
        nc.vector.memset(ones, 1.0)
        ident = const.tile([D, D], dt.bfloat16)
        make_identity(nc, ident)

        QT = persist.tile([D, NQ, S], dt.bfloat16)     # 2 MB
        KT = persist.tile([D, S], dt.bfloat16)         # 0.5 MB
        V = persist.tile([D, NKC, D], dt.bfloat16)     # 0.5 MB [s%, kc, d]
        attnT = persist.tile([D, NQ, S], dt.bfloat16)  # 2 MB

        # ---- phase 1: QKV projection + RoPE + V transpose ----
        def rope(acc_ps, out_slice, cos_t, sin_t):
            raw = tr.tile([D, SB], dt.float32, tag="raw")
            nc.scalar.copy(out=raw, in_=acc_ps)
            sw = tr.tile([D, SB], dt.float32, tag="sw")
            nc.sync.dma_start(out=sw[0:64, :], in_=raw[64:128, :])
            nc.sync.dma_start(out=sw[64:128, :], in_=raw[0:64, :])
            nc.vector.tensor_mul(out=raw, in0=raw, in1=cos_t)
            nc.vector.tensor_mul(out=sw, in0=sw, in1=sin_t)
            nc.vector.tensor_add(out=out_slice, in0=raw, in1=sw)

        for sb in range(NSB):
            ssl = slice(sb * SB, (sb + 1) * SB)
            xp = big.tile([D, NCC, SB], dt.bfloat16, tag="big")
            nc.sync.dma_start(out=xp, in_=xTr[:, :, ssl])
            cos_t = tr.tile([D, SB], dt.bfloat16, tag="cos")
            nc.sync.dma_start(out=cos_t, in_=cosT[:, ssl])
            sin_t = tr.tile([D, SB], dt.bfloat16, tag="sin")
            nc.sync.dma_start(out=sin_t, in_=sinT[:, ssl])
            qps = [ps.tile([D, SB], dt.float32, tag="ps", name=f"qps{h}")
                   for h in range(NQ)]
            kps = ps.tile([D, SB], dt.float32, tag="ps")
            vps = ps.tile([D, SB], dt.float32, tag="ps")
            for wc in range(4):       # stream weights: 8 contraction chunks
                csl = slice(wc * 8, (wc + 1) * 8)
                wq = wpool.tile([D, 8, NQ, D], dt.bfloat16, tag="wq")
                nc.sync.dma_start(out=wq, in_=wqr[:, csl])
                wk = wpool.tile([D, 8, D], dt.bfloat16, tag="wk")
                nc.sync.dma_start(out=wk, in_=wkr[:, csl])
                wv = wpool.tile([D, 8, D], dt.bfloat16, tag="wv")
                nc.sync.dma_start(out=wv, in_=wvr[:, csl])
                for cil in range(8):
                    ci = wc * 8 + cil
                    st, sp = (ci == 0), (ci == NCC - 1)
                    for h in range(NQ):
                        nc.tensor.matmul(qps[h], wq[:, cil, h], xp[:, ci],
                                         start=st, stop=sp)
                    nc.tensor.matmul(kps, wk[:, cil], xp[:, ci], start=st, stop=sp)
                    nc.tensor.matmul(vps, wv[:, cil], xp[:, ci], start=st, stop=sp)
            for h in range(NQ):
                rope(qps[h], QT[:, h, ssl], cos_t, sin_t)
            rope(kps, KT[:, ssl], cos_t, sin_t)
            vsb = tr.tile([D, SB], dt.bfloat16, tag="vsb")
            nc.scalar.copy(out=vsb, in_=vps)
            for j in range(4):
                vtp = ps.tile([D, D], dt.bfloat16, tag="rs", bufs=2)
                nc.tensor.transpose(vtp, vsb[:, j * D:(j + 1) * D], ident)
                nc.vector.tensor_copy(out=V[:, sb * 4 + j, :], in_=vtp)

        # ---- phase 2: attention ----
        for h in range(NQ):
            for qb in range(NSB):
                qsl = slice(qb * SB, (qb + 1) * SB)
                nkc = 4 * (qb + 1)          # causal: k chunks 0..4qb+3
                E = big.tile([D, NKC, SB], dt.bfloat16, tag="big")
                for kc in range(nkc):
                    stp = ps.tile([D, SB], dt.float32, tag="ps")
                    nc.tensor.matmul(stp, KT[:, kc * D:(kc + 1) * D],
                                     QT[:, h, qsl], start=True, stop=True)
                    nc.scalar.activation(out=E[:, kc, :], in_=stp,
                                         func=AF.Exp, scale=SCALE)
                    if kc >= 4 * qb:  # diagonal block: apply binary mask
                        nc.vector.tensor_mul(
                            out=E[:, kc, :], in0=E[:, kc, :],
                            in1=mask[:, qb, kc - 4 * qb, :])
                rsp = ps.tile([1, SB], dt.float32, tag="rs", bufs=2)
                pvp = ps.tile([D, SB], dt.float32, tag="ps")
                for kc in range(nkc):
                    st, sp = (kc == 0), (kc == nkc - 1)
                    nc.tensor.matmul(rsp, ones, E[:, kc, :], start=st, stop=sp)
                    nc.tensor.matmul(pvp, V[:, kc, :], E[:, kc, :],
                                     start=st, stop=sp)
                rcp = tr.tile([1, SB], dt.float32, tag="rcp")
                nc.vector.reciprocal(out=rcp, in_=rsp)
                rdr = drp.tile([1, SB], dt.float32, tag="rdr")
                nc.sync.dma_start(out=rdr, in_=rcp)
                rcpb = tr.tile([D, SB], dt.float32, tag="rcpb")
                nc.sync.dma_start(out=rcpb, in_=rdr.to_broadcast([D, SB]))
                nc.vector.tensor_mul(out=attnT[:, h, qsl], in0=pvp,
                                     in1=rcpb)

        # ---- phase 3: output projection (partial of Wo) ----
        woSb = big.tile([D, NQ, NEB, 512], dt.bfloat16, tag="big")  # 4 MB
        nc.sync.dma_start(out=woSb, in_=wor)
        for sc in range(S // D):
            scl = slice(sc * D, (sc + 1) * D)
            for eb in range(NEB):
                op = ps.tile([D, 512], dt.float32, tag="ps")
                for h in range(NQ):
                    nc.tensor.matmul(op, attnT[:, h, scl], woSb[:, h, eb],
                                     start=(h == 0), stop=(h == NQ - 1))
                osb = outp.tile([D, 512], dt.float32, tag="o")
                if eb % 2 == 0:
                    nc.scalar.copy(out=osb, in_=op)
                else:
                    nc.vector.tensor_copy(out=osb, in_=op)
                nc.sync.dma_start(
                    out=part[scl, eb * 512:(eb + 1) * 512], in_=osb)


def _prep(hidden_states, attention_mask, position_ids, Wq, Wk, Wv, Wo):
    """Host-side sharding/layout. Returns per-core input maps."""
    x = np.asarray(hidden_states, dtype=np.float32)[0]          # [S, HID]
    xT = np.ascontiguousarray(x.T).astype(BF16)                 # [HID, S]

    pos = np.asarray(position_ids)[0].astype(np.float64)        # [S]
    inv = 1.0 / (ROPE_THETA ** (np.arange(0, D, 2, dtype=np.float64) / D))
    ang = np.empty((D, S), dtype=np.float64)
    ang[:64] = inv[:, None] * pos[None, :]
    ang[64:] = ang[:64]
    cosT = np.cos(ang).astype(BF16)
    sinT = np.sin(ang)
    sinT[:64] *= -1.0                                           # sign folded
    sinT = sinT.astype(BF16)

    m = np.asarray(attention_mask, dtype=np.float32)[0, 0]      # [S, S] additive
    binT = (m > -0.5).astype(BF16).T                            # [k, q] binary
    maskTd = np.stack([
        np.ascontiguousarray(binT[j * SB:(j + 1) * SB, j * SB:(j + 1) * SB])
        for j in range(NSB)])                                   # [4, 512, 512]

    Wq = np.asarray(Wq, dtype=np.float32)
    Wk = np.asarray(Wk, dtype=np.float32)
    Wv = np.asarray(Wv, dtype=np.float32)
    Wo = np.asarray(Wo, dtype=np.float32)

    in_maps = []
    for c in range(NCORES):
        qsl = slice(c * NQ * D, (c + 1) * NQ * D)
        ksl = slice(c * D, (c + 1) * D)
        in_maps.append({
            "xT": xT,
            "wqT": np.ascontiguousarray(Wq[qsl, :].T).astype(BF16),
            "wkT": np.ascontiguousarray(Wk[ksl, :].T).astype(BF16),
            "wvT": np.ascontiguousarray(Wv[ksl, :].T).astype(BF16),
            "woT": np.ascontiguousarray(Wo[:, qsl].T).astype(BF16),
            "cosT": cosT, "sinT": sinT, "maskTd": maskTd,
        })
    return in_maps


def kernel(hidden_states, attention_mask, position_ids, Wq, Wk, Wv, Wo,
           _trace=False):
    from concourse.bass_utils import run_bass_kernel_spmd

    if "nc" not in _CACHE:
        _CACHE["nc"] = _build()
    nc = _CACHE["nc"]

    in_maps = _prep(hidden_states, attention_mask, position_ids, Wq, Wk, Wv, Wo)
    res = run_bass_kernel_spmd(nc, in_maps, core_ids=list(range(NCORES)),
                               trace=_trace)
    _CACHE["last_res"] = res
    out = res.results[0]["part"].astype(np.float64)
    for c in range(1, NCORES):
        out += res.results[c]["part"]
    return out.astype(np.float32).reshape(1, S, HID)


if __name__ == "__main__":
    pass



# revision 12
# speedup vs baseline: 1.2880x; 1.2880x over previous
"""Llama GQA attention (B=1, S=2048, HID=4096, 32 Q heads / 8 KV heads, RoPE,
causal) on 8 trn2 NeuronCores, tensor-parallel over KV heads.

Per core c: q-heads 4c..4c+3, kv-head c. Device computes a partial
out_c = attn_heads_c @ Wo[:, cols_c].T ; host sums the 8 partials (bf16).

Layout strategy (per core):
  - weights resident in SBUF (loaded once); x streamed in [128, 8, 512]
    chunks; projections QT/KT [d, s] via W_chunk.T @ xT_chunk
  - RoPE: psum freed early by batched ACT copies; half-swap via Pool-engine
    SBUF->SBUF DMA (sign folded into sin table); muls on DVE in bf16
  - attention per q-block of 512 (qb outer, head inner):
    scores(T) ST[k,q] = KT_chunk.T @ QT -> exp on ACT; diagonal 512-blocks
    column-sliced to the causal triangle (128-granular) + binary 128x128
    triangle mask; rowsum via ones-matmul (accumulated, column-sliced),
    reciprocal on DVE, partition-broadcast on Pool (no DRAM round trip);
    PV accumulated per chunk, normalized by DVE mul
  - Wo partial interleaved per q-block right after its 4 heads finish;
    bf16 [128, 4096] row-block output tiles, one DMA per 128 rows
All matmuls in bf16 with fp32 PSUM accumulation.
"""
import math

import numpy as np
import ml_dtypes

S = 2048
HID = 4096
D = 128
NQ = 4            # q heads per core
NCORES = 8
SB = 512          # s/q block
NSB = S // SB     # 4
NKC = S // D      # 16 k chunks
NEB = HID // 512  # 8 output e blocks
NCC = HID // D    # 32 contraction chunks
SCALE = 1.0 / math.sqrt(D)
ROPE_THETA = 10000.0

BF16 = ml_dtypes.bfloat16

_CACHE = {}


def _build():
    import concourse.tile as tile
    from concourse import bacc, mybir
    from concourse.masks import make_identity

    dt = mybir.dt
    nc = bacc.Bacc("TRN2", target_bir_lowering=False, debug=False,
                   num_devices=NCORES)

    xT = nc.dram_tensor("xT", [HID, S], dt.bfloat16, kind="ExternalInput")
    wqT = nc.dram_tensor("wqT", [HID, NQ * D], dt.bfloat16, kind="ExternalInput")
    wkT = nc.dram_tensor("wkT", [HID, D], dt.bfloat16, kind="ExternalInput")
    wvT = nc.dram_tensor("wvT", [HID, D], dt.bfloat16, kind="ExternalInput")
    woT = nc.dram_tensor("woT", [NQ * D, HID], dt.bfloat16, kind="ExternalInput")
    cosT = nc.dram_tensor("cosT", [D, S], dt.bfloat16, kind="ExternalInput")
    sinT = nc.dram_tensor("sinT", [D, S], dt.bfloat16, kind="ExternalInput")
    maskD = nc.dram_tensor("maskD", [D, 4, SB], dt.bfloat16,
                           kind="ExternalInput")
    part = nc.dram_tensor("part", [S, HID], dt.bfloat16, kind="ExternalOutput")

    xTr = xT.rearrange("(ko p) s -> p ko s", p=D)                 # [128,32,2048]
    wqr = wqT.rearrange("(ko p) (h d) -> p ko h d", p=D, d=D)     # [128,32,4,128]
    wkr = wkT.rearrange("(ko p) d -> p ko d", p=D)                # [128,32,128]
    wvr = wvT.rearrange("(ko p) d -> p ko d", p=D)
    wor = woT.rearrange("(h p) (eb e) -> p h eb e", p=D, e=512)   # [128,4,8,512]

    with tile.TileContext(nc) as tc:
        _body(nc, tc, tile, mybir, make_identity,
              xTr, wqr, wkr, wvr, wor, maskD, cosT, sinT, part)
    nc.compile()
    return nc


def _body(nc, tc, tile, mybir, make_identity,
          xTr, wqr, wkr, wvr, wor, maskD, cosT, sinT, part):
    from contextlib import ExitStack

    dt = mybir.dt
    AF = mybir.ActivationFunctionType

    with ExitStack() as ctx:
        const = ctx.enter_context(tc.tile_pool(name="const", bufs=1))
        persist = ctx.enter_context(tc.tile_pool(name="persist", bufs=1))
        xpool = ctx.enter_context(tc.tile_pool(name="xp", bufs=2))
        epool = ctx.enter_context(tc.tile_pool(name="ep", bufs=2))
        apool = ctx.enter_context(tc.tile_pool(name="ap", bufs=2))
        tr = ctx.enter_context(tc.tile_pool(name="tr", bufs=2))
        outp = ctx.enter_context(tc.tile_pool(name="outp", bufs=2))
        ps = ctx.enter_context(tc.tile_pool(name="ps", bufs=6, space="PSUM"))
        rs = ctx.enter_context(tc.tile_pool(name="rs", bufs=2, space="PSUM"))

        # ---- persistent weights / tables ----
        wqS = persist.tile([D, NCC, NQ, D], dt.bfloat16)   # 4 MB
        wkS = persist.tile([D, NCC, D], dt.bfloat16)       # 0.5 MB
        wvS = persist.tile([D, NCC, D], dt.bfloat16)
        woS = persist.tile([D, NQ, NEB, 512], dt.bfloat16)  # 4 MB
        QT = persist.tile([D, NQ, S], dt.bfloat16)         # 2 MB
        KT = persist.tile([D, S], dt.bfloat16)             # 0.5 MB
        V = persist.tile([D, NKC, D], dt.bfloat16)         # 0.5 MB [s%, kc, d]

        ones = const.tile([D, 1], dt.bfloat16)
        nc.vector.memset(ones, 1.0)
        ident = const.tile([D, D], dt.bfloat16)
        make_identity(nc, ident)
        mask = const.tile([D, 4, SB], dt.bfloat16)

        # ---- phase 1: QKV projection + RoPE + V transpose ----
        pending_vt = []  # deferred V transposes (vsb tile, sb index)

        def flush_vt():
            for vsb_t, sb_i in pending_vt:
                for j in range(4):
                    vtp = rs.tile([D, D], dt.bfloat16, tag="rs")
                    nc.tensor.transpose(vtp, vsb_t[:, j * D:(j + 1) * D], ident)
                    nc.vector.tensor_copy(out=V[:, sb_i * 4 + j, :], in_=vtp)
            pending_vt.clear()

        for sb in range(NSB):
            ssl = slice(sb * SB, (sb + 1) * SB)
            qps = [ps.tile([D, SB], dt.float32, tag="ps", name=f"qps{h}")
                   for h in range(NQ)]
            kps = ps.tile([D, SB], dt.float32, tag="ps")
            vps = ps.tile([D, SB], dt.float32, tag="ps")
            xch = []
            for wc in range(4):       # stream x: 8 contraction chunks per DMA
                csl = slice(wc * 8, (wc + 1) * 8)
                if sb == 0 and wc == 0:
                    # k/v weights first (small) so k/v matmuls gate early
                    nc.sync.dma_start(out=wkS, in_=wkr)
                    nc.sync.dma_start(out=wvS, in_=wvr)
                xp = xpool.tile([D, 8, SB], dt.bfloat16, tag="x")
                nc.sync.dma_start(out=xp, in_=xTr[:, csl, ssl])
                xch.append(xp)
                if sb == 0:
                    nc.sync.dma_start(out=wqS[:, csl], in_=wqr[:, csl])
                if sb == 0 and wc == 0:
                    cos_t = tr.tile([D, SB], dt.bfloat16, tag="cos")
                    nc.sync.dma_start(out=cos_t, in_=cosT[:, ssl])
                    sin_t = tr.tile([D, SB], dt.bfloat16, tag="sin")
                    nc.sync.dma_start(out=sin_t, in_=sinT[:, ssl])
                    nc.sync.dma_start(out=mask, in_=maskD[:, :, :])
            if sb > 0:
                cos_t = tr.tile([D, SB], dt.bfloat16, tag="cos")
                nc.sync.dma_start(out=cos_t, in_=cosT[:, ssl])
                sin_t = tr.tile([D, SB], dt.bfloat16, tag="sin")
                nc.sync.dma_start(out=sin_t, in_=sinT[:, ssl])
            for wc in range(4):
                for cil in range(8):
                    ci = wc * 8 + cil
                    st, sp = (ci == 0), (ci == NCC - 1)
                    nc.tensor.matmul(kps, wkS[:, ci], xch[wc][:, cil],
                                     start=st, stop=sp)
                    nc.tensor.matmul(vps, wvS[:, ci], xch[wc][:, cil],
                                     start=st, stop=sp)
                    for h in range(NQ):
                        nc.tensor.matmul(qps[h], wqS[:, ci, h], xch[wc][:, cil],
                                         start=st, stop=sp)
                if wc == 0:
                    flush_vt()  # previous sb's V transposes (PE, data ready)
            # free all 6 psum tiles ASAP with back-to-back ACT copies
            raws = []
            for h in range(NQ):
                raw = tr.tile([D, SB], dt.bfloat16, tag=f"raw{h}", bufs=1)
                nc.scalar.copy(out=raw, in_=qps[h])
                raws.append(raw)
            kraw = tr.tile([D, SB], dt.bfloat16, tag="rawk", bufs=1)
            nc.scalar.copy(out=kraw, in_=kps)
            vsb = tr.tile([D, SB], dt.bfloat16, tag="vsb", bufs=1)
            nc.scalar.copy(out=vsb, in_=vps)
            pending_vt.append((vsb, sb))

            # rope on the SBUF copies (swap halves via Pool DMA, muls on DVE)
            def rope(raw_t, out_slice):
                sw = tr.tile([D, SB], dt.bfloat16, tag="sw", bufs=3)
                nc.gpsimd.dma_start(out=sw[0:64, :], in_=raw_t[64:128, :])
                nc.gpsimd.dma_start(out=sw[64:128, :], in_=raw_t[0:64, :])
                rc = tr.tile([D, SB], dt.bfloat16, tag="rc", bufs=3)
                nc.vector.tensor_mul(out=rc, in0=raw_t, in1=cos_t)
                nc.vector.tensor_mul(out=sw, in0=sw, in1=sin_t)
                nc.vector.tensor_add(out=out_slice, in0=rc, in1=sw)

            for h in range(NQ):
                rope(raws[h], QT[:, h, ssl])
            rope(kraw, KT[:, ssl])

        woSr = wor  # issue Wo load once phase-1 input traffic is done
        nc.sync.dma_start(out=woS, in_=woSr)

        # ---- phase 2+3: attention + output projection, per q-block ----
        for qb in range(NSB):
            qsl = slice(qb * SB, (qb + 1) * SB)
            nkc = 4 * (qb + 1)          # causal: k chunks 0..4qb+3
            attn = apool.tile([D, NQ, SB], dt.bfloat16, tag="at")
            for h in range(NQ):
                E = epool.tile([D, NKC, SB], dt.bfloat16, tag="E")
                # scores + exp; diagonal chunks multiplied by the binary mask
                for kc in range(nkc):
                    stp = ps.tile([D, SB], dt.float32, tag="ps")
                    nc.tensor.matmul(stp, KT[:, kc * D:(kc + 1) * D],
                                     QT[:, h, qsl], start=True, stop=True)
                    nc.scalar.activation(out=E[:, kc, :], in_=stp,
                                         func=AF.Exp, scale=SCALE)
                    if kc >= 4 * qb:
                        j = kc - 4 * qb
                        nc.vector.tensor_mul(out=E[:, kc, :],
                                             in0=E[:, kc, :],
                                             in1=mask[:, j, :])
                if qb == 0 and h == 0:
                    flush_vt()  # sb=3 V transposes; needed from qb=3 only
                # rowsum first so recip/broadcast overlaps the PV matmuls
                rsp = rs.tile([1, SB], dt.float32, tag="rs")
                for kc in range(nkc):
                    nc.tensor.matmul(rsp, ones, E[:, kc, :],
                                     start=(kc == 0), stop=(kc == nkc - 1))
                rcp = tr.tile([1, SB], dt.float32, tag="rcp")
                nc.vector.reciprocal(out=rcp, in_=rsp)
                rcpb = tr.tile([D, SB], dt.float32, tag="rcpb")
                nc.gpsimd.partition_broadcast(rcpb, rcp)
                # PV
                pvp = ps.tile([D, SB], dt.float32, tag="ps")
                for kc in range(nkc):
                    nc.tensor.matmul(pvp, V[:, kc, :], E[:, kc, :],
                                     start=(kc == 0), stop=(kc == nkc - 1))
                nc.vector.tensor_mul(out=attn[:, h, :], in0=pvp, in1=rcpb)

            # ---- phase 3 for this q-block: partial of Wo ----
            for sc in range(4):
                scl = slice(sc * D, (sc + 1) * D)
                osc = outp.tile([D, NEB, 512], dt.bfloat16, tag="o")
                for eb in range(NEB):
                    op = ps.tile([D, 512], dt.float32, tag="ps")
                    for h in range(NQ):
                        nc.tensor.matmul(op, attn[:, h, scl], woS[:, h, eb],
                                         start=(h == 0), stop=(h == NQ - 1))
                    if eb % 2 == 0:
                        nc.scalar.copy(out=osc[:, eb], in_=op)
                    else:
                        nc.vector.tensor_copy(out=osc[:, eb], in_=op)
                g = qb * SB + sc * D
                nc.sync.dma_start(out=part[g:g + D, :], in_=osc)


def _prep(hidden_states, attention_mask, position_ids, Wq, Wk, Wv, Wo):
    """Host-side sharding/layout. Returns per-core input maps."""
    x = np.asarray(hidden_states, dtype=np.float32)[0]          # [S, HID]
    xT = np.ascontiguousarray(x.T).astype(BF16)                 # [HID, S]

    pos = np.asarray(position_ids)[0].astype(np.float64)        # [S]
    inv = 1.0 / (ROPE_THETA ** (np.arange(0, D, 2, dtype=np.float64) / D))
    ang = np.empty((D, S), dtype=np.float64)
    ang[:64] = inv[:, None] * pos[None, :]
    ang[64:] = ang[:64]
    cosT = np.cos(ang).astype(BF16)
    sinT = np.sin(ang)
    sinT[:64] *= -1.0                                           # sign folded
    sinT = sinT.astype(BF16)

    m = np.asarray(attention_mask, dtype=np.float32)[0, 0]      # [S, S] additive
    # binary mask [k%, j, q] for diagonal 512-block chunk j (causal blocks
    # are translation invariant, so one copy serves every qb)
    binT = (m > -0.5).astype(np.float32).T                      # [k, q]
    maskDv = np.ascontiguousarray(np.stack(
        [binT[j * D:(j + 1) * D, 0:SB] for j in range(4)], axis=1)).astype(BF16)

    Wq = np.asarray(Wq, dtype=np.float32)
    Wk = np.asarray(Wk, dtype=np.float32)
    Wv = np.asarray(Wv, dtype=np.float32)
    Wo = np.asarray(Wo, dtype=np.float32)

    in_maps = []
    for c in range(NCORES):
        qsl = slice(c * NQ * D, (c + 1) * NQ * D)
        ksl = slice(c * D, (c + 1) * D)
        in_maps.append({
            "xT": xT,
            "wqT": np.ascontiguousarray(Wq[qsl, :].T).astype(BF16),
            "wkT": np.ascontiguousarray(Wk[ksl, :].T).astype(BF16),
            "wvT": np.ascontiguousarray(Wv[ksl, :].T).astype(BF16),
            "woT": np.ascontiguousarray(Wo[:, qsl].T).astype(BF16),
            "cosT": cosT, "sinT": sinT, "maskD": maskDv,
        })
    return in_maps


def kernel(hidden_states, attention_mask, position_ids, Wq, Wk, Wv, Wo,
           _trace=False):
    from concourse.bass_utils import run_bass_kernel_spmd

    if "nc" not in _CACHE:
        _CACHE["nc"] = _build()
    nc = _CACHE["nc"]

    in_maps = _prep(hidden_states, attention_mask, position_ids, Wq, Wk, Wv, Wo)
    res = run_bass_kernel_spmd(nc, in_maps, core_ids=list(range(NCORES)),
                               trace=_trace)
    _CACHE["last_res"] = res
    out = res.results[0]["part"].astype(np.float64)
    for c in range(1, NCORES):
        out += res.results[c]["part"].astype(np.float64)
    return out.astype(np.float32).reshape(1, S, HID)


if __name__ == "__main__":
    pass


# revision 23
# speedup vs baseline: 1.3585x; 1.0547x over previous
"""Llama GQA attention (B=1, S=2048, HID=4096, 32 Q heads / 8 KV heads, RoPE,
causal) on 8 trn2 NeuronCores, tensor-parallel over KV heads.

Per core c: q-heads 4c..4c+3, kv-head c. Device computes a partial
out_c = attn_heads_c @ Wo[:, cols_c].T ; host sums the 8 partials (bf16).

Layout strategy (per core):
  - weights resident in SBUF (loaded once); x streamed in [128, 8, 512]
    chunks; projections QT/KT [d, s] via W_chunk.T @ xT_chunk
  - RoPE: psum freed early by batched ACT copies; half-swap via Pool-engine
    SBUF->SBUF DMA (sign folded into sin table); muls on DVE in bf16
  - attention per q-block of 512 (qb outer, head inner):
    scores(T) ST[k,q] = KT_chunk.T @ QT -> exp on ACT; diagonal 512-blocks
    column-sliced to the causal triangle (128-granular) + binary 128x128
    triangle mask; rowsum via ones-matmul (accumulated, column-sliced),
    reciprocal on DVE, partition-broadcast on Pool (no DRAM round trip);
    PV accumulated per chunk, normalized by DVE mul
  - Wo partial interleaved per q-block right after its 4 heads finish;
    bf16 [128, 4096] row-block output tiles, one DMA per 128 rows
All matmuls in bf16 with fp32 PSUM accumulation.
"""
import math

import numpy as np
import ml_dtypes

S = 2048
HID = 4096
D = 128
NQ = 4            # q heads per core
NCORES = 8
SB = 512          # s/q block
NSB = S // SB     # 4
NKC = S // D      # 16 k chunks
NEB = HID // 512  # 8 output e blocks
NCC = HID // D    # 32 contraction chunks
SCALE = 1.0 / math.sqrt(D)
ROPE_THETA = 10000.0

BF16 = ml_dtypes.bfloat16

_CACHE = {}


def _build():
    import concourse.tile as tile
    from concourse import bacc, mybir
    from concourse.masks import make_identity

    dt = mybir.dt
    nc = bacc.Bacc("TRN2", target_bir_lowering=False, debug=False,
                   num_devices=NCORES)

    xT = nc.dram_tensor("xT", [HID, S], dt.bfloat16, kind="ExternalInput")
    wqT = nc.dram_tensor("wqT", [HID, NQ * D], dt.bfloat16, kind="ExternalInput")
    wkT = nc.dram_tensor("wkT", [HID, D], dt.bfloat16, kind="ExternalInput")
    wvT = nc.dram_tensor("wvT", [HID, D], dt.bfloat16, kind="ExternalInput")
    woT = nc.dram_tensor("woT", [NQ * D, HID], dt.bfloat16, kind="ExternalInput")
    cosT = nc.dram_tensor("cosT", [D, S], dt.bfloat16, kind="ExternalInput")
    sinT = nc.dram_tensor("sinT", [D, S], dt.bfloat16, kind="ExternalInput")
    maskD = nc.dram_tensor("maskD", [D, 4, SB], dt.bfloat16,
                           kind="ExternalInput")
    part = nc.dram_tensor("part", [S, HID], dt.bfloat16, kind="ExternalOutput")

    xTr = xT.rearrange("(ko p) s -> p ko s", p=D)                 # [128,32,2048]
    wqr = wqT.rearrange("(ko p) (h d) -> p ko h d", p=D, d=D)     # [128,32,4,128]
    wkr = wkT.rearrange("(ko p) d -> p ko d", p=D)                # [128,32,128]
    wvr = wvT.rearrange("(ko p) d -> p ko d", p=D)
    wor = woT.rearrange("(h p) (eb e) -> p h eb e", p=D, e=512)   # [128,4,8,512]

    with tile.TileContext(nc) as tc:
        _body(nc, tc, tile, mybir, make_identity,
              xTr, wqr, wkr, wvr, wor, maskD, cosT, sinT, part)
    nc.compile()
    return nc


def _body(nc, tc, tile, mybir, make_identity,
          xTr, wqr, wkr, wvr, wor, maskD, cosT, sinT, part):
    from contextlib import ExitStack

    dt = mybir.dt
    AF = mybir.ActivationFunctionType

    with ExitStack() as ctx:
        const = ctx.enter_context(tc.tile_pool(name="const", bufs=1))
        persist = ctx.enter_context(tc.tile_pool(name="persist", bufs=1))
        xpool = ctx.enter_context(tc.tile_pool(name="xp", bufs=2))
        epool = ctx.enter_context(tc.tile_pool(name="ep", bufs=2))
        apool = ctx.enter_context(tc.tile_pool(name="ap", bufs=2))
        tr = ctx.enter_context(tc.tile_pool(name="tr", bufs=2))
        outp = ctx.enter_context(tc.tile_pool(name="outp", bufs=2))
        ps = ctx.enter_context(tc.tile_pool(name="ps", bufs=6, space="PSUM"))
        rs = ctx.enter_context(tc.tile_pool(name="rs", bufs=2, space="PSUM"))

        # ---- persistent weights / tables ----
        wqS = persist.tile([D, NCC, NQ, D], dt.bfloat16)   # 4 MB
        wkS = persist.tile([D, NCC, D], dt.bfloat16)       # 0.5 MB
        wvS = persist.tile([D, NCC, D], dt.bfloat16)
        woS = persist.tile([D, NQ, NEB, 512], dt.bfloat16)  # 4 MB
        QT = persist.tile([D, NQ, S], dt.bfloat16)         # 2 MB
        KT = persist.tile([D, S], dt.bfloat16)             # 0.5 MB
        V = persist.tile([D, NKC, D], dt.bfloat16)         # 0.5 MB [s%, kc, d]

        ones = const.tile([D, 1], dt.bfloat16)
        nc.vector.memset(ones, 1.0)
        ident = const.tile([D, D], dt.bfloat16)
        make_identity(nc, ident)
        mask = const.tile([D, 4, SB], dt.bfloat16)

        # ---- phase 1: QKV projection + RoPE + V transpose ----
        pending_vt = []  # deferred V transposes (vsb tile, sb index)

        def flush_vt():
            for vsb_t, sb_i in pending_vt:
                for j in range(4):
                    vtp = rs.tile([D, D], dt.bfloat16, tag="rs")
                    nc.tensor.transpose(vtp, vsb_t[:, j * D:(j + 1) * D], ident)
                    # ACT, not DVE: DVE is clogged with rope muls at sb
                    # boundaries, which would stall the PE transposes
                    nc.scalar.copy(out=V[:, sb_i * 4 + j, :], in_=vtp)
            pending_vt.clear()

        for sb in range(NSB):
            ssl = slice(sb * SB, (sb + 1) * SB)
            qps = [ps.tile([D, SB], dt.float32, tag="ps", name=f"qps{h}")
                   for h in range(NQ)]
            kps = ps.tile([D, SB], dt.float32, tag="ps")
            vps = ps.tile([D, SB], dt.float32, tag="ps")
            xch = []
            for wc in range(4):       # stream x: 8 contraction chunks per DMA
                csl = slice(wc * 8, (wc + 1) * 8)
                if sb == 0 and wc == 0:
                    # halved first chunk so the very first matmul gates on
                    # 0.5 MB instead of 1 MB
                    xpa = xpool.tile([D, 4, SB], dt.bfloat16, tag="x")
                    nc.sync.dma_start(out=xpa, in_=xTr[:, 0:4, ssl])
                    xpb = xpool.tile([D, 4, SB], dt.bfloat16, tag="xb", bufs=1)
                    nc.sync.dma_start(out=xpb, in_=xTr[:, 4:8, ssl])
                    xch.append((xpa, xpb))
                else:
                    xp = xpool.tile([D, 8, SB], dt.bfloat16, tag="x")
                    nc.sync.dma_start(out=xp, in_=xTr[:, csl, ssl])
                    xch.append(xp)
                if sb == 0:
                    nc.sync.dma_start(out=wkS[:, csl], in_=wkr[:, csl])
                    nc.sync.dma_start(out=wvS[:, csl], in_=wvr[:, csl])
                    nc.sync.dma_start(out=wqS[:, csl], in_=wqr[:, csl])
                if sb == 0 and wc == 0:
                    cos_t = tr.tile([D, SB], dt.bfloat16, tag="cos")
                    nc.sync.dma_start(out=cos_t, in_=cosT[:, ssl])
                    sin_t = tr.tile([D, SB], dt.bfloat16, tag="sin")
                    nc.sync.dma_start(out=sin_t, in_=sinT[:, ssl])
                    nc.sync.dma_start(out=mask, in_=maskD[:, :, :])
            if sb > 0:
                cos_t = tr.tile([D, SB], dt.bfloat16, tag="cos")
                nc.sync.dma_start(out=cos_t, in_=cosT[:, ssl])
                sin_t = tr.tile([D, SB], dt.bfloat16, tag="sin")
                nc.sync.dma_start(out=sin_t, in_=sinT[:, ssl])

            for wc in range(4):
                if sb == 0 and wc == 0:
                    # first chunk: group k, then v, then q so early matmuls
                    # gate on the small wk/wv chunks, not on wq
                    for cil in range(8):
                        nc.tensor.matmul(kps, wkS[:, cil], xch[0][:, cil],
                                         start=(cil == 0), stop=False)
                    for cil in range(8):
                        nc.tensor.matmul(vps, wvS[:, cil], xch[0][:, cil],
                                         start=(cil == 0), stop=False)
                    for cil in range(8):
                        for h in range(NQ):
                            nc.tensor.matmul(qps[h], wqS[:, cil, h],
                                             xch[0][:, cil],
                                             start=(cil == 0), stop=False)
                    continue
                for cil in range(8):
                    ci = wc * 8 + cil
                    st, sp = (ci == 0), (ci == NCC - 1)
                    nc.tensor.matmul(kps, wkS[:, ci], xch[wc][:, cil],
                                     start=st, stop=sp)
                    nc.tensor.matmul(vps, wvS[:, ci], xch[wc][:, cil],
                                     start=st, stop=sp)
                    for h in range(NQ):
                        nc.tensor.matmul(qps[h], wqS[:, ci, h], xch[wc][:, cil],
                                         start=st, stop=sp)
                if wc == 0:
                    flush_vt()  # previous sb's V transposes (PE, data ready)
            # free all 6 psum tiles ASAP with back-to-back ACT copies
            # free psums in the order the next consumer reuses the ring:
            # next sb's stream starts k,v,q0.. ; phase 2's stp ring reuses
            # slots in allocation order (qps0..3, kps, vps)
            kraw = vsb = None
            raws = []

            def copy_kv():
                nonlocal kraw, vsb
                kraw = tr.tile([D, SB], dt.bfloat16, tag="rawk", bufs=1)
                nc.scalar.copy(out=kraw, in_=kps)
                vsb = tr.tile([D, SB], dt.bfloat16, tag="vsb", bufs=1)
                nc.scalar.copy(out=vsb, in_=vps)

            if sb < NSB - 1:
                copy_kv()
            for h in range(NQ):
                raw = tr.tile([D, SB], dt.bfloat16, tag=f"raw{h}", bufs=1)
                nc.scalar.copy(out=raw, in_=qps[h])
                raws.append(raw)
            if sb == NSB - 1:
                copy_kv()
            pending_vt.append((vsb, sb))

            # rope on the SBUF copies (swap halves via Pool DMA, muls on DVE)
            def rope(raw_t, out_slice):
                sw = tr.tile([D, SB], dt.bfloat16, tag="sw", bufs=3)
                nc.gpsimd.dma_start(out=sw[0:64, :], in_=raw_t[64:128, :])
                nc.gpsimd.dma_start(out=sw[64:128, :], in_=raw_t[0:64, :])
                rc = tr.tile([D, SB], dt.bfloat16, tag="rc", bufs=3)
                nc.vector.tensor_mul(out=rc, in0=raw_t, in1=cos_t)
                nc.vector.tensor_mul(out=sw, in0=sw, in1=sin_t)
                nc.vector.tensor_add(out=out_slice, in0=rc, in1=sw)

            for h in range(NQ):
                rope(raws[h], QT[:, h, ssl])
            rope(kraw, KT[:, ssl])

        woSr = wor  # issue Wo load once phase-1 input traffic is done
        nc.sync.dma_start(out=woS, in_=woSr)

        # ---- phase 2+3: attention + output projection, per q-block ----
        def ph3(qb3, attn3):
            for sc in range(4):
                scl = slice(sc * D, (sc + 1) * D)
                osc = outp.tile([D, NEB, 512], dt.bfloat16, tag="o")
                for eb in range(NEB):
                    op = ps.tile([D, 512], dt.float32, tag="ps")
                    for h in range(NQ):
                        nc.tensor.matmul(op, attn3[:, h, scl], woS[:, h, eb],
                                         start=(h == 0), stop=(h == NQ - 1))
                    if eb % 2 == 0:
                        nc.scalar.copy(out=osc[:, eb], in_=op)
                    else:
                        nc.vector.tensor_copy(out=osc[:, eb], in_=op)
                g = qb3 * SB + sc * D
                # two half DMAs: the first half ships while eb 4-7 copy
                nc.sync.dma_start(out=part[g:g + D, 0:HID // 2],
                                  in_=osc[:, 0:NEB // 2])
                nc.sync.dma_start(out=part[g:g + D, HID // 2:HID],
                                  in_=osc[:, NEB // 2:NEB])

        prev_attn = None
        for qb in range(NSB):
            qsl = slice(qb * SB, (qb + 1) * SB)
            nkc = 4 * (qb + 1)          # causal: k chunks 0..4qb+3
            attn = apool.tile([D, NQ, SB], dt.bfloat16, tag="at")
            Es = {}

            def scores(h):
                E = epool.tile([D, NKC, SB], dt.bfloat16, tag="E")
                Es[h] = E
                # scores + exp; diagonal chunks multiplied by the binary mask
                for kc in range(nkc):
                    stp = ps.tile([D, SB], dt.float32, tag="ps")
                    nc.tensor.matmul(stp, KT[:, kc * D:(kc + 1) * D],
                                     QT[:, h, qsl], start=True, stop=True)
                    nc.scalar.activation(out=E[:, kc, :], in_=stp,
                                         func=AF.Exp, scale=SCALE)
                    if kc >= 4 * qb:
                        j = kc - 4 * qb
                        nc.vector.tensor_mul(out=E[:, kc, :],
                                             in0=E[:, kc, :],
                                             in1=mask[:, j, :])

            def finish(h):
                E = Es.pop(h)
                # rowsum first so recip/broadcast overlaps the PV matmuls
                rsp = rs.tile([1, SB], dt.float32, tag="rs")
                for kc in range(nkc):
                    nc.tensor.matmul(rsp, ones, E[:, kc, :],
                                     start=(kc == 0), stop=(kc == nkc - 1))
                rcp = tr.tile([1, SB], dt.float32, tag="rcp")
                nc.vector.reciprocal(out=rcp, in_=rsp)
                rcpb = tr.tile([D, SB], dt.float32, tag="rcpb")
                nc.gpsimd.partition_broadcast(rcpb, rcp)
                # PV
                pvp = ps.tile([D, SB], dt.float32, tag="ps")
                for kc in range(nkc):
                    nc.tensor.matmul(pvp, V[:, kc, :], E[:, kc, :],
                                     start=(kc == 0), stop=(kc == nkc - 1))
                nc.vector.tensor_mul(out=attn[:, h, :], in0=pvp, in1=rcpb)

            # pipeline heads one stage deep: scores(h+1) issues before the
            # rowsum/PV of h, so exp/recip/broadcast latency hides under PE;
            # the previous q-block's Wo projection is emitted after scores(0)
            # so the final head's normalize chain hides under its matmuls
            scores(0)
            if qb == 0:
                flush_vt()  # sb=3 V transposes; needed from qb=3 only
            else:
                ph3(qb - 1, prev_attn)
            for h in range(1, NQ):
                scores(h)
                finish(h - 1)
            finish(NQ - 1)
            prev_attn = attn
        ph3(NSB - 1, prev_attn)


def _prep(hidden_states, attention_mask, position_ids, Wq, Wk, Wv, Wo):
    """Host-side sharding/layout. Returns per-core input maps."""
    x = np.asarray(hidden_states, dtype=np.float32)[0]          # [S, HID]
    xT = np.ascontiguousarray(x.T).astype(BF16)                 # [HID, S]

    pos = np.asarray(position_ids)[0].astype(np.float64)        # [S]
    inv = 1.0 / (ROPE_THETA ** (np.arange(0, D, 2, dtype=np.float64) / D))
    ang = np.empty((D, S), dtype=np.float64)
    ang[:64] = inv[:, None] * pos[None, :]
    ang[64:] = ang[:64]
    cosT = np.cos(ang).astype(BF16)
    sinT = np.sin(ang)
    sinT[:64] *= -1.0                                           # sign folded
    sinT = sinT.astype(BF16)

    m = np.asarray(attention_mask, dtype=np.float32)[0, 0]      # [S, S] additive
    # binary mask [k%, j, q] for diagonal 512-block chunk j (causal blocks
    # are translation invariant, so one copy serves every qb)
    binT = (m > -0.5).astype(np.float32).T                      # [k, q]
    maskDv = np.ascontiguousarray(np.stack(
        [binT[j * D:(j + 1) * D, 0:SB] for j in range(4)], axis=1)).astype(BF16)

    Wq = np.asarray(Wq, dtype=np.float32)
    Wk = np.asarray(Wk, dtype=np.float32)
    Wv = np.asarray(Wv, dtype=np.float32)
    Wo = np.asarray(Wo, dtype=np.float32)

    in_maps = []
    for c in range(NCORES):
        qsl = slice(c * NQ * D, (c + 1) * NQ * D)
        ksl = slice(c * D, (c + 1) * D)
        in_maps.append({
            "xT": xT,
            "wqT": np.ascontiguousarray(Wq[qsl, :].T).astype(BF16),
            "wkT": np.ascontiguousarray(Wk[ksl, :].T).astype(BF16),
            "wvT": np.ascontiguousarray(Wv[ksl, :].T).astype(BF16),
            "woT": np.ascontiguousarray(Wo[:, qsl].T).astype(BF16),
            "cosT": cosT, "sinT": sinT, "maskD": maskDv,
        })
    return in_maps


def kernel(hidden_states, attention_mask, position_ids, Wq, Wk, Wv, Wo,
           _trace=False):
    from concourse.bass_utils import run_bass_kernel_spmd

    if "nc" not in _CACHE:
        _CACHE["nc"] = _build()
    nc = _CACHE["nc"]

    in_maps = _prep(hidden_states, attention_mask, position_ids, Wq, Wk, Wv, Wo)
    res = run_bass_kernel_spmd(nc, in_maps, core_ids=list(range(NCORES)),
                               trace=_trace)
    _CACHE["last_res"] = res
    out = res.results[0]["part"].astype(np.float64)
    for c in range(1, NCORES):
        out += res.results[c]["part"].astype(np.float64)
    return out.astype(np.float32).reshape(1, S, HID)


if __name__ == "__main__":
    pass


# revision 30
# speedup vs baseline: 1.4006x; 1.0310x over previous
"""Llama GQA attention (B=1, S=2048, HID=4096, 32 Q heads / 8 KV heads, RoPE,
causal) on 8 trn2 NeuronCores, tensor-parallel over KV heads.

Per core c: q-heads 4c..4c+3, kv-head c. Device computes a partial
out_c = attn_heads_c @ Wo[:, cols_c].T ; host sums the 8 partials (bf16).

Layout strategy (per core):
  - weights resident in SBUF (loaded once); x streamed in [128, 8, 512]
    chunks; projections QT/KT [d, s] via W_chunk.T @ xT_chunk
  - RoPE: psum freed early by batched ACT copies; half-swap via Pool-engine
    SBUF->SBUF DMA (sign folded into sin table); muls on DVE in bf16
  - attention per q-block of 512 (qb outer, head inner):
    scores(T) ST[k,q] = KT_chunk.T @ QT -> exp on ACT; diagonal 512-blocks
    column-sliced to the causal triangle (128-granular) + binary 128x128
    triangle mask; rowsum via ones-matmul (accumulated, column-sliced),
    reciprocal on DVE, partition-broadcast on Pool (no DRAM round trip);
    PV accumulated per chunk, normalized by DVE mul
  - Wo partial interleaved per q-block right after its 4 heads finish;
    bf16 [128, 4096] row-block output tiles, one DMA per 128 rows
All matmuls in bf16 with fp32 PSUM accumulation.
"""
import math

import numpy as np
import ml_dtypes

S = 2048
HID = 4096
D = 128
NQ = 4            # q heads per core
NCORES = 8
SB = 512          # s/q block
NSB = S // SB     # 4
NKC = S // D      # 16 k chunks
NEB = HID // 512  # 8 output e blocks
NCC = HID // D    # 32 contraction chunks
SCALE = 1.0 / math.sqrt(D)
ROPE_THETA = 10000.0

BF16 = ml_dtypes.bfloat16

_CACHE = {}


def _build():
    import concourse.tile as tile
    from concourse import bacc, mybir
    from concourse.masks import make_identity

    dt = mybir.dt
    nc = bacc.Bacc("TRN2", target_bir_lowering=False, debug=False,
                   num_devices=NCORES)

    xT = nc.dram_tensor("xT", [HID, S], dt.bfloat16, kind="ExternalInput")
    wqT = nc.dram_tensor("wqT", [HID, NQ * D], dt.bfloat16, kind="ExternalInput")
    wkT = nc.dram_tensor("wkT", [HID, D], dt.bfloat16, kind="ExternalInput")
    wvT = nc.dram_tensor("wvT", [HID, D], dt.bfloat16, kind="ExternalInput")
    woT = nc.dram_tensor("woT", [NQ * D, HID], dt.bfloat16, kind="ExternalInput")
    cosT = nc.dram_tensor("cosT", [D, S], dt.bfloat16, kind="ExternalInput")
    sinT = nc.dram_tensor("sinT", [D, S], dt.bfloat16, kind="ExternalInput")
    maskD = nc.dram_tensor("maskD", [D, 4, SB], dt.bfloat16,
                           kind="ExternalInput")
    part = nc.dram_tensor("part", [S, HID], dt.bfloat16, kind="ExternalOutput")

    xTr = xT.rearrange("(ko p) s -> p ko s", p=D)                 # [128,32,2048]
    wqr = wqT.rearrange("(ko p) (h d) -> p ko h d", p=D, d=D)     # [128,32,4,128]
    wkr = wkT.rearrange("(ko p) d -> p ko d", p=D)                # [128,32,128]
    wvr = wvT.rearrange("(ko p) d -> p ko d", p=D)
    wor = woT.rearrange("(h p) (eb e) -> p h eb e", p=D, e=512)   # [128,4,8,512]

    with tile.TileContext(nc) as tc:
        _body(nc, tc, tile, mybir, make_identity,
              xTr, wqr, wkr, wvr, wor, maskD, cosT, sinT, part)
    nc.compile()
    return nc


def _body(nc, tc, tile, mybir, make_identity,
          xTr, wqr, wkr, wvr, wor, maskD, cosT, sinT, part):
    from contextlib import ExitStack

    dt = mybir.dt
    AF = mybir.ActivationFunctionType

    with ExitStack() as ctx:
        const = ctx.enter_context(tc.tile_pool(name="const", bufs=1))
        persist = ctx.enter_context(tc.tile_pool(name="persist", bufs=1))
        xpool = ctx.enter_context(tc.tile_pool(name="xp", bufs=2))
        epool = ctx.enter_context(tc.tile_pool(name="ep", bufs=2))
        apool = ctx.enter_context(tc.tile_pool(name="ap", bufs=2))
        tr = ctx.enter_context(tc.tile_pool(name="tr", bufs=2))
        outp = ctx.enter_context(tc.tile_pool(name="outp", bufs=2))
        ps = ctx.enter_context(tc.tile_pool(name="ps", bufs=6, space="PSUM"))
        rs = ctx.enter_context(tc.tile_pool(name="rs", bufs=2, space="PSUM"))

        # ---- persistent weights / tables ----
        wqS = persist.tile([D, NCC, NQ, D], dt.bfloat16)   # 4 MB
        wkS = persist.tile([D, NCC, D], dt.bfloat16)       # 0.5 MB
        wvS = persist.tile([D, NCC, D], dt.bfloat16)
        woS = persist.tile([D, NQ, NEB, 512], dt.bfloat16)  # 4 MB
        QT = persist.tile([D, NQ, S], dt.bfloat16)         # 2 MB
        KT = persist.tile([D, S], dt.bfloat16)             # 0.5 MB
        V = persist.tile([D, NKC, D], dt.bfloat16)         # 0.5 MB [s%, kc, d]

        ones = const.tile([D, 1], dt.bfloat16)
        nc.vector.memset(ones, 1.0)
        ident = const.tile([D, D], dt.bfloat16)
        make_identity(nc, ident)
        mask = const.tile([D, 4, SB], dt.bfloat16)

        # ---- phase 1: QKV projection + RoPE + V transpose ----
        pending_vt = []  # deferred V transposes (vsb tile, sb index)

        def flush_vt():
            for vsb_t, sb_i in pending_vt:
                for j in range(4):
                    vtp = rs.tile([D, D], dt.bfloat16, tag="rs")
                    nc.tensor.transpose(vtp, vsb_t[:, j * D:(j + 1) * D], ident)
                    # ACT, not DVE: DVE is clogged with rope muls at sb
                    # boundaries, which would stall the PE transposes
                    nc.scalar.copy(out=V[:, sb_i * 4 + j, :], in_=vtp)
            pending_vt.clear()

        for sb in range(NSB):
            ssl = slice(sb * SB, (sb + 1) * SB)
            qps = [ps.tile([D, SB], dt.float32, tag="ps", name=f"qps{h}")
                   for h in range(NQ)]
            # last sb: k/v psums go on the rs ring so phase 2's first score
            # tiles find two ps-ring slots already free
            kvp = rs if sb == NSB - 1 else ps
            kps = kvp.tile([D, SB], dt.float32, tag="rs" if sb == NSB - 1 else "ps")
            vps = kvp.tile([D, SB], dt.float32, tag="rs" if sb == NSB - 1 else "ps")
            xch = []
            for wc in range(4):       # stream x: 8 contraction chunks per DMA
                csl = slice(wc * 8, (wc + 1) * 8)
                if sb == 0 and wc == 0:
                    # small k/v weight chunks first, then a halved first x
                    # chunk, so the very first matmuls gate on 0.75 MB
                    nc.sync.dma_start(out=wkS[:, csl], in_=wkr[:, csl])
                    nc.sync.dma_start(out=wvS[:, csl], in_=wvr[:, csl])
                    xpa = xpool.tile([D, 4, SB], dt.bfloat16, tag="x")
                    nc.sync.dma_start(out=xpa, in_=xTr[:, 0:4, ssl])
                    xpb = xpool.tile([D, 4, SB], dt.bfloat16, tag="xb", bufs=1)
                    nc.sync.dma_start(out=xpb, in_=xTr[:, 4:8, ssl])
                    xch.append((xpa, xpb))
                    nc.sync.dma_start(out=wqS[:, csl], in_=wqr[:, csl])
                else:
                    xp = xpool.tile([D, 8, SB], dt.bfloat16, tag="x")
                    nc.sync.dma_start(out=xp, in_=xTr[:, csl, ssl])
                    xch.append(xp)
                    if sb == 0:
                        nc.sync.dma_start(out=wkS[:, csl], in_=wkr[:, csl])
                        nc.sync.dma_start(out=wvS[:, csl], in_=wvr[:, csl])
                        nc.sync.dma_start(out=wqS[:, csl], in_=wqr[:, csl])
                if sb == 0 and wc == 0:
                    cos_t = tr.tile([D, SB], dt.bfloat16, tag="cos")
                    nc.sync.dma_start(out=cos_t, in_=cosT[:, ssl])
                    sin_t = tr.tile([D, SB], dt.bfloat16, tag="sin")
                    nc.sync.dma_start(out=sin_t, in_=sinT[:, ssl])
                    nc.sync.dma_start(out=mask, in_=maskD[:, :, :])
            if sb > 0:
                cos_t = tr.tile([D, SB], dt.bfloat16, tag="cos")
                nc.sync.dma_start(out=cos_t, in_=cosT[:, ssl])
                sin_t = tr.tile([D, SB], dt.bfloat16, tag="sin")
                nc.sync.dma_start(out=sin_t, in_=sinT[:, ssl])

            for wc in range(4):
                if sb == 0 and wc == 0:
                    # first chunk: group k, then v, then q so early matmuls
                    # gate on the small wk/wv chunks, not on wq
                    xpa, xpb = xch[0]

                    def x0(cil):
                        return xpa[:, cil] if cil < 4 else xpb[:, cil - 4]

                    for cil in range(8):
                        nc.tensor.matmul(kps, wkS[:, cil], x0(cil),
                                         start=(cil == 0), stop=False)
                    for cil in range(8):
                        nc.tensor.matmul(vps, wvS[:, cil], x0(cil),
                                         start=(cil == 0), stop=False)
                    for cil in range(8):
                        for h in range(NQ):
                            nc.tensor.matmul(qps[h], wqS[:, cil, h], x0(cil),
                                             start=(cil == 0), stop=False)
                    continue
                for cil in range(8):
                    ci = wc * 8 + cil
                    st, sp = (ci == 0), (ci == NCC - 1)
                    nc.tensor.matmul(kps, wkS[:, ci], xch[wc][:, cil],
                                     start=st, stop=sp)
                    nc.tensor.matmul(vps, wvS[:, ci], xch[wc][:, cil],
                                     start=st, stop=sp)
                    for h in range(NQ):
                        nc.tensor.matmul(qps[h], wqS[:, ci, h], xch[wc][:, cil],
                                         start=st, stop=sp)
                if wc == 0:
                    flush_vt()  # previous sb's V transposes (PE, data ready)
            # free all 6 psum tiles ASAP with back-to-back ACT copies
            # free psums in the order the next consumer reuses the ring:
            # next sb's stream starts k,v,q0.. ; phase 2's stp ring reuses
            # slots in allocation order (qps0..3, kps, vps)
            kraw = vsb = None
            raws = []

            def copy_kv():
                nonlocal kraw, vsb
                kraw = tr.tile([D, SB], dt.bfloat16, tag="rawk", bufs=1)
                nc.scalar.copy(out=kraw, in_=kps)
                vsb = tr.tile([D, SB], dt.bfloat16, tag="vsb", bufs=1)
                nc.scalar.copy(out=vsb, in_=vps)

            if sb < NSB - 1:
                copy_kv()
            for h in range(NQ):
                raw = tr.tile([D, SB], dt.bfloat16, tag=f"raw{h}", bufs=1)
                nc.scalar.copy(out=raw, in_=qps[h])
                raws.append(raw)
            if sb == NSB - 1:
                copy_kv()
            pending_vt.append((vsb, sb))

            # rope on the SBUF copies (swap halves via Pool DMA, muls on DVE)
            def rope(raw_t, out_slice):
                sw = tr.tile([D, SB], dt.bfloat16, tag="sw", bufs=2)
                nc.gpsimd.dma_start(out=sw[0:64, :], in_=raw_t[64:128, :])
                nc.gpsimd.dma_start(out=sw[64:128, :], in_=raw_t[0:64, :])
                rc = tr.tile([D, SB], dt.bfloat16, tag="rc", bufs=3)
                nc.vector.tensor_mul(out=rc, in0=raw_t, in1=cos_t)
                nc.vector.tensor_mul(out=sw, in0=sw, in1=sin_t)
                nc.vector.tensor_add(out=out_slice, in0=rc, in1=sw)

            for h in range(NQ):
                rope(raws[h], QT[:, h, ssl])
            rope(kraw, KT[:, ssl])

        woSr = wor  # issue Wo load once phase-1 input traffic is done
        nc.sync.dma_start(out=woS, in_=woSr)

        # ---- phase 2+3: attention + output projection, per q-block ----
        def ph3(qb3, attn3):
            for sc in range(4):
                scl = slice(sc * D, (sc + 1) * D)
                osc = outp.tile([D, NEB, 512], dt.bfloat16, tag="o")
                for eb in range(NEB):
                    op = ps.tile([D, 512], dt.float32, tag="ps")
                    for h in range(NQ):
                        nc.tensor.matmul(op, attn3[:, h, scl], woS[:, h, eb],
                                         start=(h == 0), stop=(h == NQ - 1))
                    if eb % 2 == 0:
                        nc.scalar.copy(out=osc[:, eb], in_=op)
                    else:
                        nc.vector.tensor_copy(out=osc[:, eb], in_=op)
                g = qb3 * SB + sc * D
                # two half DMAs: the first half ships while eb 4-7 copy
                nc.sync.dma_start(out=part[g:g + D, 0:HID // 2],
                                  in_=osc[:, 0:NEB // 2])
                nc.sync.dma_start(out=part[g:g + D, HID // 2:HID],
                                  in_=osc[:, NEB // 2:NEB])

        prev_attn = None
        prev_qb = None
        # qb 1 first (its QT/KT blocks are ready long before phase-1's tail
        # ropes finish); latency-bound qb 0 last, hidden under ph3(3)
        for qb in (1, 2, 3, 0):
            qsl = slice(qb * SB, (qb + 1) * SB)
            nkc = 4 * (qb + 1)          # causal: k chunks 0..4qb+3
            attn = apool.tile([D, NQ, SB], dt.bfloat16, tag="at")
            Es = {}

            def scores(h):
                E = epool.tile([D, NKC, SB], dt.bfloat16, tag="E")
                Es[h] = E
                # scores + exp; diagonal chunks multiplied by the binary mask
                for kc in range(nkc):
                    stp = ps.tile([D, SB], dt.float32, tag="ps")
                    nc.tensor.matmul(stp, KT[:, kc * D:(kc + 1) * D],
                                     QT[:, h, qsl], start=True, stop=True)
                    nc.scalar.activation(out=E[:, kc, :], in_=stp,
                                         func=AF.Exp, scale=SCALE)
                    if kc >= 4 * qb:
                        j = kc - 4 * qb
                        nc.vector.tensor_mul(out=E[:, kc, :],
                                             in0=E[:, kc, :],
                                             in1=mask[:, j, :])

            def finish(h):
                E = Es.pop(h)
                # rowsum first so recip/broadcast overlaps the PV matmuls
                rsp = rs.tile([1, SB], dt.float32, tag="rs")
                for kc in range(nkc):
                    nc.tensor.matmul(rsp, ones, E[:, kc, :],
                                     start=(kc == 0), stop=(kc == nkc - 1))
                rcp = tr.tile([1, SB], dt.float32, tag="rcp")
                nc.vector.reciprocal(out=rcp, in_=rsp)
                rcpb = tr.tile([D, SB], dt.float32, tag="rcpb")
                nc.gpsimd.partition_broadcast(rcpb, rcp)
                # PV
                pvp = ps.tile([D, SB], dt.float32, tag="ps")
                for kc in range(nkc):
                    nc.tensor.matmul(pvp, V[:, kc, :], E[:, kc, :],
                                     start=(kc == 0), stop=(kc == nkc - 1))
                nc.vector.tensor_mul(out=attn[:, h, :], in0=pvp, in1=rcpb)

            # pipeline heads one stage deep: scores(h+1) issues before the
            # rowsum/PV of h, so exp/recip/broadcast latency hides under PE;
            # the previous q-block's Wo projection is emitted after scores(0)
            # so the final head's normalize chain hides under its matmuls
            scores(0)
            if prev_attn is None:
                flush_vt()  # sb=3 V transposes; needed for qb=3's PV
            else:
                ph3(prev_qb, prev_attn)
            for h in range(1, NQ):
                scores(h)
                finish(h - 1)
            finish(NQ - 1)
            prev_attn = attn
            prev_qb = qb
        ph3(prev_qb, prev_attn)


def _prep(hidden_states, attention_mask, position_ids, Wq, Wk, Wv, Wo):
    """Host-side sharding/layout. Returns per-core input maps."""
    x = np.asarray(hidden_states, dtype=np.float32)[0]          # [S, HID]
    xT = np.ascontiguousarray(x.T).astype(BF16)                 # [HID, S]

    pos = np.asarray(position_ids)[0].astype(np.float64)        # [S]
    inv = 1.0 / (ROPE_THETA ** (np.arange(0, D, 2, dtype=np.float64) / D))
    ang = np.empty((D, S), dtype=np.float64)
    ang[:64] = inv[:, None] * pos[None, :]
    ang[64:] = ang[:64]
    cosT = np.cos(ang).astype(BF16)
    sinT = np.sin(ang)
    sinT[:64] *= -1.0                                           # sign folded
    sinT = sinT.astype(BF16)

    m = np.asarray(attention_mask, dtype=np.float32)[0, 0]      # [S, S] additive
    # binary mask [k%, j, q] for diagonal 512-block chunk j (causal blocks
    # are translation invariant, so one copy serves every qb)
    binT = (m > -0.5).astype(np.float32).T                      # [k, q]
    maskDv = np.ascontiguousarray(np.stack(
        [binT[j * D:(j + 1) * D, 0:SB] for j in range(4)], axis=1)).astype(BF16)

    Wq = np.asarray(Wq, dtype=np.float32)
    Wk = np.asarray(Wk, dtype=np.float32)
    Wv = np.asarray(Wv, dtype=np.float32)
    Wo = np.asarray(Wo, dtype=np.float32)

    in_maps = []
    for c in range(NCORES):
        qsl = slice(c * NQ * D, (c + 1) * NQ * D)
        ksl = slice(c * D, (c + 1) * D)
        in_maps.append({
            "xT": xT,
            "wqT": np.ascontiguousarray(Wq[qsl, :].T).astype(BF16),
            "wkT": np.ascontiguousarray(Wk[ksl, :].T).astype(BF16),
            "wvT": np.ascontiguousarray(Wv[ksl, :].T).astype(BF16),
            "woT": np.ascontiguousarray(Wo[:, qsl].T).astype(BF16),
            "cosT": cosT, "sinT": sinT, "maskD": maskDv,
        })
    return in_maps


def kernel(hidden_states, attention_mask, position_ids, Wq, Wk, Wv, Wo,
           _trace=False):
    from concourse.bass_utils import run_bass_kernel_spmd

    if "nc" not in _CACHE:
        _CACHE["nc"] = _build()
    nc = _CACHE["nc"]

    in_maps = _prep(hidden_states, attention_mask, position_ids, Wq, Wk, Wv, Wo)
    res = run_bass_kernel_spmd(nc, in_maps, core_ids=list(range(NCORES)),
                               trace=_trace)
    _CACHE["last_res"] = res
    out = res.results[0]["part"].astype(np.float64)
    for c in range(1, NCORES):
        out += res.results[c]["part"].astype(np.float64)
    return out.astype(np.float32).reshape(1, S, HID)


if __name__ == "__main__":
    pass


# revision 37
# speedup vs baseline: 1.4185x; 1.0128x over previous
"""Llama GQA attention (B=1, S=2048, HID=4096, 32 Q heads / 8 KV heads, RoPE,
causal) on 8 trn2 NeuronCores, tensor-parallel over KV heads.

Per core c: q-heads 4c..4c+3, kv-head c. Device computes a partial
out_c = attn_heads_c @ Wo[:, cols_c].T ; host sums the 8 partials (bf16).

Layout strategy (per core):
  - weights resident in SBUF (loaded once); x streamed in [128, 8, 512]
    chunks; projections QT/KT [d, s] via W_chunk.T @ xT_chunk
  - RoPE: psum freed early by batched ACT copies; half-swap via Pool-engine
    SBUF->SBUF DMA (sign folded into sin table); muls on DVE in bf16
  - attention per q-block of 512 (qb outer, head inner):
    scores(T) ST[k,q] = KT_chunk.T @ QT -> exp on ACT; diagonal 512-blocks
    column-sliced to the causal triangle (128-granular) + binary 128x128
    triangle mask; rowsum via ones-matmul (accumulated, column-sliced),
    reciprocal on DVE, partition-broadcast on Pool (no DRAM round trip);
    PV accumulated per chunk, normalized by DVE mul
  - Wo partial interleaved per q-block right after its 4 heads finish;
    bf16 [128, 4096] row-block output tiles, one DMA per 128 rows
All matmuls in bf16 with fp32 PSUM accumulation.
"""
import math

import numpy as np
import ml_dtypes

S = 2048
HID = 4096
D = 128
NQ = 4            # q heads per core
NCORES = 8
SB = 512          # s/q block
NSB = S // SB     # 4
NKC = S // D      # 16 k chunks
NEB = HID // 512  # 8 output e blocks
NCC = HID // D    # 32 contraction chunks
SCALE = 1.0 / math.sqrt(D)
ROPE_THETA = 10000.0

BF16 = ml_dtypes.bfloat16

_CACHE = {}


def _build():
    import concourse.tile as tile
    from concourse import bacc, mybir
    from concourse.masks import make_identity

    dt = mybir.dt
    nc = bacc.Bacc("TRN2", target_bir_lowering=False, debug=False,
                   num_devices=NCORES)

    xT = nc.dram_tensor("xT", [HID, S], dt.bfloat16, kind="ExternalInput")
    wqT = nc.dram_tensor("wqT", [HID, NQ * D], dt.bfloat16, kind="ExternalInput")
    wkT = nc.dram_tensor("wkT", [HID, D], dt.bfloat16, kind="ExternalInput")
    wvT = nc.dram_tensor("wvT", [HID, D], dt.bfloat16, kind="ExternalInput")
    woT = nc.dram_tensor("woT", [NQ * D, HID], dt.bfloat16, kind="ExternalInput")
    cosT = nc.dram_tensor("cosT", [D, S], dt.bfloat16, kind="ExternalInput")
    sinT = nc.dram_tensor("sinT", [D, S], dt.bfloat16, kind="ExternalInput")
    maskD = nc.dram_tensor("maskD", [D, 4, SB], dt.bfloat16,
                           kind="ExternalInput")
    part = nc.dram_tensor("part", [S, HID], dt.bfloat16, kind="ExternalOutput")

    xTr = xT.rearrange("(ko p) s -> p ko s", p=D)                 # [128,32,2048]
    wqr = wqT.rearrange("(ko p) (h d) -> p ko h d", p=D, d=D)     # [128,32,4,128]
    wkr = wkT.rearrange("(ko p) d -> p ko d", p=D)                # [128,32,128]
    wvr = wvT.rearrange("(ko p) d -> p ko d", p=D)
    wor = woT.rearrange("(h p) (eb e) -> p h eb e", p=D, e=512)   # [128,4,8,512]

    with tile.TileContext(nc) as tc:
        _body(nc, tc, tile, mybir, make_identity,
              xTr, wqr, wkr, wvr, wor, maskD, cosT, sinT, part)
    nc.compile()
    return nc


def _body(nc, tc, tile, mybir, make_identity,
          xTr, wqr, wkr, wvr, wor, maskD, cosT, sinT, part):
    from contextlib import ExitStack

    dt = mybir.dt
    AF = mybir.ActivationFunctionType

    with ExitStack() as ctx:
        const = ctx.enter_context(tc.tile_pool(name="const", bufs=1))
        persist = ctx.enter_context(tc.tile_pool(name="persist", bufs=1))
        xpool = ctx.enter_context(tc.tile_pool(name="xp", bufs=2))
        epool = ctx.enter_context(tc.tile_pool(name="ep", bufs=2))
        apool = ctx.enter_context(tc.tile_pool(name="ap", bufs=2))
        tr = ctx.enter_context(tc.tile_pool(name="tr", bufs=2))
        outp = ctx.enter_context(tc.tile_pool(name="outp", bufs=2))
        ps = ctx.enter_context(tc.tile_pool(name="ps", bufs=6, space="PSUM"))
        rs = ctx.enter_context(tc.tile_pool(name="rs", bufs=2, space="PSUM"))

        # ---- persistent weights / tables ----
        wqS = persist.tile([D, NCC, NQ, D], dt.bfloat16)   # 4 MB
        wkS = persist.tile([D, NCC, D], dt.bfloat16)       # 0.5 MB
        wvS = persist.tile([D, NCC, D], dt.bfloat16)
        woS = persist.tile([D, NQ, NEB, 512], dt.bfloat16)  # 4 MB
        QT = persist.tile([D, NQ, S], dt.bfloat16)         # 2 MB
        KT = persist.tile([D, S], dt.bfloat16)             # 0.5 MB
        V = persist.tile([D, NKC, D], dt.bfloat16)         # 0.5 MB [s%, kc, d]

        ones = const.tile([D, 1], dt.bfloat16)
        nc.vector.memset(ones, 1.0)
        ident = const.tile([D, D], dt.bfloat16)
        make_identity(nc, ident)
        mask = const.tile([D, 4, SB], dt.bfloat16)

        # ---- phase 1: QKV projection + RoPE + V transpose ----
        pending_vt = []  # deferred V transposes (vsb tile, sb index)

        def flush_vt(on_dve=False):
            for vsb_t, sb_i in pending_vt:
                for j in range(4):
                    vtp = rs.tile([D, D], dt.bfloat16, tag="rs")
                    nc.tensor.transpose(vtp, vsb_t[:, j * D:(j + 1) * D], ident)
                    # mid-phase-1: ACT (DVE is clogged with rope muls);
                    # at qb3: DVE (ACT is clogged with the 16-chunk exps)
                    if on_dve:
                        nc.vector.tensor_copy(out=V[:, sb_i * 4 + j, :],
                                              in_=vtp)
                    else:
                        nc.scalar.copy(out=V[:, sb_i * 4 + j, :], in_=vtp)
            pending_vt.clear()

        for sb in range(NSB):
            ssl = slice(sb * SB, (sb + 1) * SB)
            qps = [ps.tile([D, SB], dt.float32, tag="ps", name=f"qps{h}")
                   for h in range(NQ)]
            # last sb: k/v psums go on the rs ring so phase 2's first score
            # tiles find two ps-ring slots already free
            kvp = rs if sb == NSB - 1 else ps
            kps = kvp.tile([D, SB], dt.float32, tag="rs" if sb == NSB - 1 else "ps")
            vps = kvp.tile([D, SB], dt.float32, tag="rs" if sb == NSB - 1 else "ps")
            xch = []
            for wc in range(4):       # stream x: 8 contraction chunks per DMA
                csl = slice(wc * 8, (wc + 1) * 8)
                if sb == 0 and wc == 0:
                    # tiny first weight piece + halved first x chunk, so the
                    # very first matmul gates on ~0.5 MB
                    nc.sync.dma_start(out=wkS[:, 0:1], in_=wkr[:, 0:1])
                    xpa = xpool.tile([D, 4, SB], dt.bfloat16, tag="x")
                    nc.sync.dma_start(out=xpa, in_=xTr[:, 0:4, ssl])
                    nc.sync.dma_start(out=wkS[:, 1:8], in_=wkr[:, 1:8])
                    nc.sync.dma_start(out=wvS[:, csl], in_=wvr[:, csl])
                    xpb = xpool.tile([D, 4, SB], dt.bfloat16, tag="xb", bufs=1)
                    nc.sync.dma_start(out=xpb, in_=xTr[:, 4:8, ssl])
                    xch.append((xpa, xpb))
                    nc.sync.dma_start(out=wqS[:, csl], in_=wqr[:, csl])
                else:
                    xp = xpool.tile([D, 8, SB], dt.bfloat16, tag="x")
                    nc.sync.dma_start(out=xp, in_=xTr[:, csl, ssl])
                    xch.append(xp)
                    if sb == 0:
                        nc.sync.dma_start(out=wkS[:, csl], in_=wkr[:, csl])
                        nc.sync.dma_start(out=wvS[:, csl], in_=wvr[:, csl])
                        nc.sync.dma_start(out=wqS[:, csl], in_=wqr[:, csl])
                if sb == 0 and wc == 0:
                    cos_t = tr.tile([D, SB], dt.bfloat16, tag="cos")
                    nc.sync.dma_start(out=cos_t, in_=cosT[:, ssl])
                    sin_t = tr.tile([D, SB], dt.bfloat16, tag="sin")
                    nc.sync.dma_start(out=sin_t, in_=sinT[:, ssl])
                    nc.sync.dma_start(out=mask, in_=maskD[:, :, :])
            if sb > 0:
                cos_t = tr.tile([D, SB], dt.bfloat16, tag="cos")
                nc.sync.dma_start(out=cos_t, in_=cosT[:, ssl])
                sin_t = tr.tile([D, SB], dt.bfloat16, tag="sin")
                nc.sync.dma_start(out=sin_t, in_=sinT[:, ssl])

            for wc in range(4):
                if sb == 0 and wc == 0:
                    # first chunk: group k, then v, then q so early matmuls
                    # gate on the small wk/wv chunks, not on wq
                    xpa, xpb = xch[0]

                    def x0(cil):
                        return xpa[:, cil] if cil < 4 else xpb[:, cil - 4]

                    for cil in range(8):
                        nc.tensor.matmul(kps, wkS[:, cil], x0(cil),
                                         start=(cil == 0), stop=False)
                    for cil in range(8):
                        nc.tensor.matmul(vps, wvS[:, cil], x0(cil),
                                         start=(cil == 0), stop=False)
                    for cil in range(8):
                        for h in range(NQ):
                            nc.tensor.matmul(qps[h], wqS[:, cil, h], x0(cil),
                                             start=(cil == 0), stop=False)
                    continue
                for cil in range(8):
                    ci = wc * 8 + cil
                    st, sp = (ci == 0), (ci == NCC - 1)
                    nc.tensor.matmul(kps, wkS[:, ci], xch[wc][:, cil],
                                     start=st, stop=sp)
                    nc.tensor.matmul(vps, wvS[:, ci], xch[wc][:, cil],
                                     start=st, stop=sp)
                    for h in range(NQ):
                        nc.tensor.matmul(qps[h], wqS[:, ci, h], xch[wc][:, cil],
                                         start=st, stop=sp)
                if wc == 0:
                    flush_vt()  # previous sb's V transposes (PE, data ready)
            # free all 6 psum tiles ASAP with back-to-back ACT copies
            # free psums in the order the next consumer reuses the ring:
            # next sb's stream starts k,v,q0.. ; phase 2's stp ring reuses
            # slots in allocation order (qps0..3, kps, vps)
            kraw = vsb = None
            raws = []

            def copy_kv():
                nonlocal kraw, vsb
                kraw = tr.tile([D, SB], dt.bfloat16, tag="rawk", bufs=1)
                nc.scalar.copy(out=kraw, in_=kps)
                vsb = tr.tile([D, SB], dt.bfloat16, tag="vsb", bufs=1)
                nc.scalar.copy(out=vsb, in_=vps)

            if sb < NSB - 1:
                copy_kv()
            for h in range(NQ):
                raw = tr.tile([D, SB], dt.bfloat16, tag=f"raw{h}", bufs=1)
                if sb == NSB - 1:
                    # DVE, so ACT is free to run phase 2's first exps the
                    # moment their score matmuls land
                    nc.vector.tensor_copy(out=raw, in_=qps[h])
                else:
                    nc.scalar.copy(out=raw, in_=qps[h])
                raws.append(raw)
            if sb == NSB - 1:
                copy_kv()
            pending_vt.append((vsb, sb))

            # rope on the SBUF copies (swap halves via Pool DMA, muls on DVE)
            def rope(raw_t, out_slice):
                sw = tr.tile([D, SB], dt.bfloat16, tag="sw", bufs=2)
                nc.gpsimd.dma_start(out=sw[0:64, :], in_=raw_t[64:128, :])
                nc.gpsimd.dma_start(out=sw[64:128, :], in_=raw_t[0:64, :])
                rc = tr.tile([D, SB], dt.bfloat16, tag="rc", bufs=3)
                nc.vector.tensor_mul(out=rc, in0=raw_t, in1=cos_t)
                nc.vector.tensor_mul(out=sw, in0=sw, in1=sin_t)
                nc.vector.tensor_add(out=out_slice, in0=rc, in1=sw)

            for h in range(NQ):
                rope(raws[h], QT[:, h, ssl])
            rope(kraw, KT[:, ssl])

        woSr = wor  # issue Wo load once phase-1 input traffic is done
        nc.sync.dma_start(out=woS, in_=woSr)

        # ---- phase 2+3: attention + output projection, per q-block ----
        def ph3(qb3, attn3):
            for sc in range(4):
                scl = slice(sc * D, (sc + 1) * D)
                osc = outp.tile([D, NEB, 512], dt.bfloat16, tag="o")
                for eb in range(NEB):
                    op = ps.tile([D, 512], dt.float32, tag="ps")
                    for h in range(NQ):
                        nc.tensor.matmul(op, attn3[:, h, scl], woS[:, h, eb],
                                         start=(h == 0), stop=(h == NQ - 1))
                    if eb % 2 == 0:
                        nc.scalar.copy(out=osc[:, eb], in_=op)
                    else:
                        nc.vector.tensor_copy(out=osc[:, eb], in_=op)
                g = qb3 * SB + sc * D
                # split DMAs: earlier pieces ship while later ebs still copy
                npc = 4 if (qb3 == 0 and sc == 3) else 2
                w = NEB // npc
                for p in range(npc):
                    nc.sync.dma_start(
                        out=part[g:g + D, p * w * 512:(p + 1) * w * 512],
                        in_=osc[:, p * w:(p + 1) * w])

        prev_attn = None
        prev_qb = None
        # qb 1 first (its QT/KT blocks are ready long before phase-1's tail
        # ropes finish); latency-bound qb 0 last, hidden under ph3(3)
        for qb in (1, 2, 3, 0):
            qsl = slice(qb * SB, (qb + 1) * SB)
            nkc = 4 * (qb + 1)          # causal: k chunks 0..4qb+3
            attn = apool.tile([D, NQ, SB], dt.bfloat16, tag="at")
            Es = {}

            def scores(h):
                E = epool.tile([D, NKC, SB], dt.bfloat16, tag="E")
                Es[h] = E
                # scores + exp; diagonal chunks multiplied by the binary mask
                for kc in range(nkc):
                    stp = ps.tile([D, SB], dt.float32, tag="ps")
                    nc.tensor.matmul(stp, KT[:, kc * D:(kc + 1) * D],
                                     QT[:, h, qsl], start=True, stop=True)
                    nc.scalar.activation(out=E[:, kc, :], in_=stp,
                                         func=AF.Exp, scale=SCALE)
                    if kc >= 4 * qb:
                        j = kc - 4 * qb
                        nc.vector.tensor_mul(out=E[:, kc, :],
                                             in0=E[:, kc, :],
                                             in1=mask[:, j, :])

            def finish(h):
                E = Es.pop(h)
                # rowsum first so recip/broadcast overlaps the PV matmuls
                rsp = rs.tile([1, SB], dt.float32, tag="rs")
                for kc in range(nkc):
                    nc.tensor.matmul(rsp, ones, E[:, kc, :],
                                     start=(kc == 0), stop=(kc == nkc - 1))
                rcp = tr.tile([1, SB], dt.float32, tag="rcp")
                nc.vector.reciprocal(out=rcp, in_=rsp)
                rcpb = tr.tile([D, SB], dt.float32, tag="rcpb")
                nc.gpsimd.partition_broadcast(rcpb, rcp)
                # PV
                pvp = ps.tile([D, SB], dt.float32, tag="ps")
                for kc in range(nkc):
                    nc.tensor.matmul(pvp, V[:, kc, :], E[:, kc, :],
                                     start=(kc == 0), stop=(kc == nkc - 1))
                nc.vector.tensor_mul(out=attn[:, h, :], in0=pvp, in1=rcpb)

            # pipeline heads one stage deep: scores(h+1) issues before the
            # rowsum/PV of h, so exp/recip/broadcast latency hides under PE;
            # the previous q-block's Wo projection is emitted after scores(0)
            # so the final head's normalize chain hides under its matmuls
            scores(0)
            if qb == 3:
                # sb=3 V transposes deferred to here (first use is qb3's PV)
                flush_vt(on_dve=True)
            if prev_attn is not None:
                ph3(prev_qb, prev_attn)
            for h in range(1, NQ):
                scores(h)
                finish(h - 1)
            finish(NQ - 1)
            prev_attn = attn
            prev_qb = qb
        ph3(prev_qb, prev_attn)


def _prep(hidden_states, attention_mask, position_ids, Wq, Wk, Wv, Wo):
    """Host-side sharding/layout. Returns per-core input maps."""
    x = np.asarray(hidden_states, dtype=np.float32)[0]          # [S, HID]
    xT = np.ascontiguousarray(x.T).astype(BF16)                 # [HID, S]

    pos = np.asarray(position_ids)[0].astype(np.float64)        # [S]
    inv = 1.0 / (ROPE_THETA ** (np.arange(0, D, 2, dtype=np.float64) / D))
    ang = np.empty((D, S), dtype=np.float64)
    ang[:64] = inv[:, None] * pos[None, :]
    ang[64:] = ang[:64]
    cosT = np.cos(ang).astype(BF16)
    sinT = np.sin(ang)
    sinT[:64] *= -1.0                                           # sign folded
    sinT = sinT.astype(BF16)

    m = np.asarray(attention_mask, dtype=np.float32)[0, 0]      # [S, S] additive
    # binary mask [k%, j, q] for diagonal 512-block chunk j (causal blocks
    # are translation invariant, so one copy serves every qb)
    binT = (m > -0.5).astype(np.float32).T                      # [k, q]
    maskDv = np.ascontiguousarray(np.stack(
        [binT[j * D:(j + 1) * D, 0:SB] for j in range(4)], axis=1)).astype(BF16)

    Wq = np.asarray(Wq, dtype=np.float32)
    Wk = np.asarray(Wk, dtype=np.float32)
    Wv = np.asarray(Wv, dtype=np.float32)
    Wo = np.asarray(Wo, dtype=np.float32)

    in_maps = []
    for c in range(NCORES):
        qsl = slice(c * NQ * D, (c + 1) * NQ * D)
        ksl = slice(c * D, (c + 1) * D)
        in_maps.append({
            "xT": xT,
            "wqT": np.ascontiguousarray(Wq[qsl, :].T).astype(BF16),
            "wkT": np.ascontiguousarray(Wk[ksl, :].T).astype(BF16),
            "wvT": np.ascontiguousarray(Wv[ksl, :].T).astype(BF16),
            "woT": np.ascontiguousarray(Wo[:, qsl].T).astype(BF16),
            "cosT": cosT, "sinT": sinT, "maskD": maskDv,
        })
    return in_maps


def kernel(hidden_states, attention_mask, position_ids, Wq, Wk, Wv, Wo,
           _trace=False):
    from concourse.bass_utils import run_bass_kernel_spmd

    if "nc" not in _CACHE:
        _CACHE["nc"] = _build()
    nc = _CACHE["nc"]

    in_maps = _prep(hidden_states, attention_mask, position_ids, Wq, Wk, Wv, Wo)
    res = run_bass_kernel_spmd(nc, in_maps, core_ids=list(range(NCORES)),
                               trace=_trace)
    _CACHE["last_res"] = res
    out = res.results[0]["part"].astype(np.float64)
    for c in range(1, NCORES):
        out += res.results[c]["part"].astype(np.float64)
    return out.astype(np.float32).reshape(1, S, HID)


if __name__ == "__main__":
    pass
